# revision 42
# baseline (speedup 1.0000x reference)
"""CLIP loss kernel for trn2, 8 NeuronCores, data-parallel over the batch dim.

Strategy (per core c of 8, SPMD):
  inputs: img slice [1024, 512] f32, spec slice [1024, 512] f32 (rows
  1024c..1024c+1023 of each modality).
  1. sumsq of both slices on ACT (Square + accum_out); 16/|row| =
     Sqrt(256 * reciprocal(ss)) (DVE reciprocal + ACT Sqrt). ACT uses
     exactly two table sets (sqrt preamble / exp main loop), both
     loaded off the critical path via warm-up activations.
  2. both modalities normalized on-device to bf16 (x16 fp8 prescale
     folded into the Sqrt scale), transposed via PE, cast to fp8 in the
     PSUM->SBUF staging copies. spec^T AllGathered in two chunks (chunk
     0 gates only the first half of the main loop; the first mesh
     cannot begin before the CC core's ~50-65us init anyway). The
     collective payload keeps rows = SBUF partitions, and specT uses an
     arrival-major layout [q, src core, k, off], so every unpack DMA
     moves 2KB-contiguous runs on both sides (the column permutation is
     harmless: the host only ever sums over all columns).
  3. logits block: out[m=img rows, n=spec cols] = imgT.T @ specT, fp8
     DoubleRow (K=256 per pass), PSUM f32, tiles [128, 2048].
  4. ACT Exp with constant scale = logit_scale/256; accum_out gives
     row-sums of exp for free. m=0 exp tile writes straight into
     racc[128, 8192] (bf16 column partials); m>0 tiles accumulated via
     DVE add. Final 128-partition column reduce happens on host.
  5. diag: raw img.spec dot per row (DVE, in the collective's shadow),
     combined with norms on host.
Host: gathers per-core row-sums / column partials / diag pieces, takes
logs and means (O(N) numpy) -> scalar loss.
"""

import os
from contextlib import ExitStack

import numpy as np

import concourse.bass as bass
import concourse.mybir as mybir
from concourse import bacc, tile
from concourse.bass_utils import run_bass_kernel_spmd
from concourse.masks import make_identity

N, D, C = 8192, 512, 8
NL = N // C  # 1024 local rows per core
P = 128
T = NL // P  # 8 natural [128, 512] tiles per modality slice
KC = D // P  # 4 contraction chunks
G = 4        # column groups per core block
GW = N // G  # 2048 columns per group

f32 = mybir.dt.float32
bf16 = mybir.dt.bfloat16
fp8 = mybir.dt.float8e4
FA = mybir.ActivationFunctionType
ALU = mybir.AluOpType

# fp8 operands are pre-scaled by 16 to stay out of the subnormal range;
# both sides carry the factor, compensated by scale/256 in the exp.
FP8_PRESCALE = 16.0

_cache: dict = {}

# dev bisection knobs (shipping defaults; NOTE tensor_tensor_reduce and
# fp8 PE transpose both fail on hardware — do not reintroduce them)
_USE_FP8T = os.environ.get("KERNEL_FP8T", "0") == "1"
_USE_DUMMY_CC = os.environ.get("KERNEL_DUMMY_CC", "0") == "1"


def _build(scale: float, use_cc: bool = True):
    nc = bacc.Bacc("TRN2", target_bir_lowering=False, debug=False, num_devices=C)
    img = nc.dram_tensor("img", [NL, D], bf16, kind="ExternalInput")
    spec = nc.dram_tensor("spec", [NL, D], bf16, kind="ExternalInput")
    rowsum_o = nc.dram_tensor("rowsum", [P, T], f32, kind="ExternalOutput")
    racc_o = nc.dram_tensor("racc_o", [P, N], bf16, kind="ExternalOutput")
    dotd_o = nc.dram_tensor("dotd", [P, T], f32, kind="ExternalOutput")
    rni_o = nc.dram_tensor("rni", [P, T], f32, kind="ExternalOutput")
    rns_o = nc.dram_tensor("rns", [P, T], f32, kind="ExternalOutput")

    tdt = fp8 if _USE_FP8T else bf16

    with tile.TileContext(nc) as tc, ExitStack() as ctx:
        const = ctx.enter_context(tc.tile_pool(name="const", bufs=1))
        natp = ctx.enter_context(tc.tile_pool(name="nat", bufs=T))
        scp = ctx.enter_context(tc.tile_pool(name="scr", bufs=2))
        spn = ctx.enter_context(tc.tile_pool(name="specn", bufs=4))
        pers = ctx.enter_context(tc.tile_pool(name="pers", bufs=1))
        ps = ctx.enter_context(tc.tile_pool(name="ps", bufs=2, space="PSUM"))
        ep = ctx.enter_context(tc.tile_pool(name="e", bufs=4))
        dramp = ctx.enter_context(tc.tile_pool(name="dram", bufs=1, space="DRAM"))

        if use_cc and _USE_DUMMY_CC:
            # First instruction of the program: trigger a tiny collective so
            # the CC core's ~30-40us init pipeline starts immediately. The
            # first real mesh cannot begin before that init completes.
            dmy_i = dramp.tile([1, 128], fp8, name="dmy_i")
            dmy_o = dramp.tile([C, 128], fp8, addr_space="Shared", name="dmy_o")
            nc.gpsimd.collective_compute(
                "AllGather",
                ALU.bypass,
                replica_groups=[list(range(C))],
                ins=[dmy_i.opt()],
                outs=[dmy_o.opt()],
            )

        ident_f = const.tile([P, P], f32, name="identf")
        make_identity(nc, ident_f)
        ident_t = const.tile([P, P], tdt, name="identt")
        nc.vector.tensor_copy(ident_t, ident_f)

        imgT = pers.tile([P, T, KC, P], fp8, name="imgT")
        # gathered spec^T in arrival-major layout [q, src core, k, off]:
        # both unpack DMA sides are then 2KB-contiguous per partition
        specT = pers.tile([P, 2, C, KC, 512], fp8, name="specT")
        stage = [pers.tile([P, NL], fp8, name=f"stage{k}") for k in range(KC)]
        racc = pers.tile([P, N], bf16, name="racc")
        # one extra slot: the split second half of the very last tile
        rowacc = pers.tile([P, T, G + 1], f32, name="rowacc")
        ssi = pers.tile([P, T], f32, name="ssi")
        sss = pers.tile([P, T], f32, name="sss")
        rsi = pers.tile([P, T], f32, name="rsi")
        rss = pers.tile([P, T], f32, name="rss")
        rni16 = pers.tile([P, T], f32, name="rni16")
        rns16 = pers.tile([P, T], f32, name="rns16")
        dotd = pers.tile([P, T], f32, name="dotd")
        rows = pers.tile([P, T], f32, name="rows")

        # two chunked AllGathers: the mesh has a ~14us floor and meshes
        # serialize, but chunk 0 gates only the first two column groups.
        # Payload rows = SBUF partitions so the unpack runs are contiguous.
        cc_in = [dramp.tile([P, KC * 512], fp8, name=f"cc_in{q}") for q in range(2)]
        cc_out = [
            dramp.tile([C * P, KC * 512], fp8, addr_space="Shared", name=f"cc_out{q}")
            for q in range(2)
        ]

        # preload the sqrt table set before the first Square needs it
        warm = const.tile([P, 1], f32, name="actwarm")
        nc.vector.memset(warm, 1.0)
        nc.scalar.activation(warm, warm, FA.Sqrt)

        # ---- per chunk (2 tiles): load -> norms -> normalize(fp8) ->
        #      transpose -> DMA PSUM->cc_in -> AllGather.
        #      img loads are deferred so the spec DMAs go first.
        img_nat, spec_nat = [], [None] * T
        for th in range(2):
            hs = slice(4 * th, 4 * th + 4)
            for tt in range(4):
                t = 4 * th + tt
                st = natp.tile([P, D], bf16, tag="specnat")
                nc.sync.dma_start(st, spec.ap()[t * P : (t + 1) * P, :])
                spec_nat[t] = st
                s2 = scp.tile([P, D], f32, tag="scr")
                nc.scalar.activation(
                    s2, st, FA.Square, accum_out=sss[:, t : t + 1]
                )
            nc.vector.tensor_scalar_max(sss[:, hs], sss[:, hs], 1.0e-6)
            nc.vector.reciprocal(rss[:, hs], sss[:, hs])
            # 16/|row|: sqrt(256 * 1/ss)
            nc.scalar.activation(
                rns16[:, hs], rss[:, hs], FA.Sqrt, scale=FP8_PRESCALE**2
            )
            pt = ps.tile([P, 2048], tdt, tag="mm")
            for tt in range(4):
                t = 4 * th + tt
                sn = spn.tile([P, D], tdt, tag="specn")
                nc.vector.tensor_scalar_mul(sn, spec_nat[t], rns16[:, t : t + 1])
                for k in range(KC):
                    nc.tensor.transpose(
                        pt[:, 512 * k + 128 * tt : 512 * k + 128 * (tt + 1)],
                        sn[:, 128 * k : 128 * (k + 1)],
                        ident_t,
                    )
            for k in range(KC):
                nc.vector.tensor_copy(
                    stage[k][:, 512 * th : 512 * (th + 1)],
                    pt[:, 512 * k : 512 * (k + 1)],
                )
                nc.sync.dma_start(
                    cc_in[th][:, 512 * k : 512 * (k + 1)],
                    stage[k][:, 512 * th : 512 * (th + 1)],
                )
            if use_cc:
                nc.gpsimd.collective_compute(
                    "AllGather",
                    ALU.bypass,
                    replica_groups=[list(range(C))],
                    ins=[cc_in[th].opt()],
                    outs=[cc_out[th].opt()],
                )

        for t in range(T):
            it = natp.tile([P, D], bf16, tag="imgnat")
            nc.sync.dma_start(it, img.ap()[t * P : (t + 1) * P, :])
            img_nat.append(it)

        # ---- img norms + transpose (overlap the collective) ----
        for t in range(T):
            s1 = scp.tile([P, D], f32, tag="scr")
            nc.scalar.activation(
                s1, img_nat[t], FA.Square, accum_out=ssi[:, t : t + 1]
            )
        nc.vector.tensor_scalar_max(ssi, ssi, 1.0e-6)
        nc.vector.reciprocal(rsi, ssi)
        nc.scalar.activation(rni16, rsi, FA.Sqrt, scale=FP8_PRESCALE**2)
        for t in range(T):
            ig = spn.tile([P, D], tdt, tag="specn")
            nc.vector.tensor_scalar_mul(ig, img_nat[t], rni16[:, t : t + 1])
            pti = ps.tile([P, 512], tdt, tag="mm")
            for k in range(KC):
                nc.tensor.transpose(
                    pti[:, 128 * k : 128 * (k + 1)],
                    ig[:, 128 * k : 128 * (k + 1)],
                    ident_t,
                )
            # ACT does this copy: it idles during the collective window
            nc.scalar.copy(imgT[:, t, :, :], pti)

        # switch the ACT table set to exp while the collective runs.
        # Reading rni16 (written by the last Sqrt) pins this after the
        # sqrt-set activations so the scheduler cannot hoist it earlier.
        warm2 = const.tile([P, 1], f32, name="actwarm2")
        nc.scalar.activation(warm2, rni16[:, 0:1], FA.Exp, scale=-1.0)

        # diag dots on DVE fill the collective's shadow (raw operands)
        for t in range(T):
            s3 = scp.tile([P, D], f32, tag="scr")
            nc.vector.tensor_mul(out=s3, in0=img_nat[t], in1=spec_nat[t])
            nc.vector.reduce_sum(
                dotd[:, t : t + 1], s3, axis=mybir.AxisListType.X
            )

        # ---- load gathered spec^T. specT[:, q, r, k, off] <-> global spec
        # row 1024*r + 512*q + off (host only ever sums over all columns, so
        # the column permutation needs no host-side handling). One DMA per
        # (q, r): 2KB-contiguous runs on both sides.
        for q in range(2):
            for r in range(C):
                if use_cc:
                    src = cc_out[q][P * r : P * (r + 1), :]
                else:  # debug: replicate the local slice (numerically wrong)
                    src = cc_in[q][:, :]
                nc.sync.dma_start(specT[:, q, r, :, :], src)

        # ---- main loop: logits block, exp, row/col accumulation ----
        nc.vector.memset(rowacc[:, :, G : G + 1], 0.0)
        escale = scale / (FP8_PRESCALE * FP8_PRESCALE)
        with nc.allow_low_precision("bf16 exp-sum accumulation, error ~0.5% -> <1e-3 on loss"):
            for g in range(G):
                gsl = racc[:, GW * g : GW * (g + 1)]
                for m in range(T):
                    pm = ps.tile([P, GW], f32, tag="mm")
                    # fp8 DoubleRow: each matmul contracts 2 k-chunks (K=256)
                    for q in range(KC // 2):
                        for ns in range(GW // 512):
                            b = 4 * g + ns  # 512-col block = (qq, r) of specT
                            nc.tensor.matmul(
                                pm[:, 512 * ns : 512 * (ns + 1)],
                                imgT[:, m, 2 * q : 2 * q + 2, :],
                                specT[:, b // C, b % C, 2 * q : 2 * q + 2, :],
                                start=(q == 0),
                                stop=(q == KC // 2 - 1),
                                perf_mode=mybir.MatmulPerfMode.DoubleRow,
                            )
                    if m == 0:
                        nc.scalar.activation(
                            gsl, pm, FA.Exp,
                            scale=escale,
                            accum_out=rowacc[:, m, g : g + 1],
                        )
                    elif g == G - 1 and m == T - 1:
                        # last tile: exp/add/store pipeline in halves so the
                        # kernel tail is ~2us shorter
                        for h in range(2):
                            hsl = slice(1024 * h, 1024 * (h + 1))
                            e = ep.tile([P, 1024], bf16, tag="e")
                            nc.scalar.activation(
                                e, pm[:, hsl], FA.Exp,
                                scale=escale,
                                accum_out=rowacc[:, m, g + h : g + h + 1],
                            )
                            nc.vector.tensor_add(
                                out=gsl[:, hsl], in0=gsl[:, hsl], in1=e
                            )
                            nc.sync.dma_start(
                                racc_o.ap()[:, GW * g + 1024 * h :
                                            GW * g + 1024 * (h + 1)],
                                gsl[:, hsl],
                            )
                    else:
                        e = ep.tile([P, GW], bf16, tag="e")
                        nc.scalar.activation(
                            e, pm, FA.Exp,
                            scale=escale,
                            accum_out=rowacc[:, m, g : g + 1],
                        )
                        nc.vector.tensor_add(out=gsl, in0=gsl, in1=e)
                if not (g == G - 1):
                    # racc[g] complete: ship it out now, overlapping next g
                    nc.sync.dma_start(
                        racc_o.ap()[:, GW * g : GW * (g + 1)], gsl
                    )

        # ---- tails ----
        nc.vector.reduce_sum(rows, rowacc[:, :, :], axis=mybir.AxisListType.X)
        nc.sync.dma_start(rowsum_o.ap(), rows)
        nc.sync.dma_start(dotd_o.ap(), dotd)
        nc.sync.dma_start(rni_o.ap(), rni16)
        nc.sync.dma_start(rns_o.ap(), rns16)

    nc.compile()
    return nc


def _ensure_ntff_hook():
    """antenv.axon_hooks is absent on this image; provide the tiny get/set
    registry and register trn_agent_boot's ctypes NTFF hook so trace=True
    works. Only used from test runs (KERNEL_TRACE=1)."""
    import sys
    import types

    try:
        import antenv.axon_hooks  # noqa: F401
        return
    except ImportError:
        pass
    mod = types.ModuleType("antenv.axon_hooks")
    _state = {"hook": None}
    mod.set_axon_ntff_profile_hook = lambda h: _state.__setitem__("hook", h)
    mod.get_axon_ntff_profile_hook = lambda: _state["hook"]
    import antenv

    sys.modules["antenv.axon_hooks"] = mod
    antenv.axon_hooks = mod
    try:
        from trn_agent_boot.trn_boot import _ntff_profile_via_ctypes

        mod.set_axon_ntff_profile_hook(
            _ntff_profile_via_ctypes("/opt/axon/libaxon_pjrt.so")
        )
    except Exception as e:  # degrade to no tracing
        print(f"NTFF hook setup failed: {e}")


def kernel(image_features, spectrum_features, logit_scale):
    scale = float(np.asarray(logit_scale))
    key = round(scale, 9)
    if key not in _cache:
        _cache[key] = _build(scale)
    nc = _cache[key]

    import ml_dtypes

    img = np.ascontiguousarray(
        np.asarray(image_features, dtype=np.float32).astype(ml_dtypes.bfloat16)
    )
    spec = np.ascontiguousarray(
        np.asarray(spectrum_features, dtype=np.float32).astype(ml_dtypes.bfloat16)
    )
    in_maps = [
        {"img": img[c * NL : (c + 1) * NL], "spec": spec[c * NL : (c + 1) * NL]}
        for c in range(C)
    ]
    trace = os.environ.get("KERNEL_TRACE") == "1"
    if trace:
        _ensure_ntff_hook()
    res = run_bass_kernel_spmd(nc, in_maps, core_ids=list(range(C)), trace=trace)
    if trace:
        print(f"HW exec time: {res.exec_time_ns} ns (mean {res.mean_exec_time_ns})")

    rs = np.stack([r["rowsum"] for r in res.results]).astype(np.float64)   # [C,P,T]
    cs = np.stack(
        [r["racc_o"].astype(np.float64).sum(axis=0) for r in res.results]
    )  # [C,N]
    dd = np.stack([r["dotd"] for r in res.results]).astype(np.float64)
    ri = np.stack([r["rni"] for r in res.results]).astype(np.float64)
    rr = np.stack([r["rns"] for r in res.results]).astype(np.float64)

    # rni/rns outputs carry the x16 fp8 prescale each
    diag_sum = float(np.sum(scale * dd * ri * rr)) / (FP8_PRESCALE * FP8_PRESCALE)
    lse_i_sum = float(np.sum(np.log(rs)))
    col_total = cs.sum(axis=0)  # still in device (chunk-major) column order
    lse_s_sum = float(np.sum(np.log(col_total)))
    loss = 0.5 * ((lse_i_sum - diag_sum) / N + (lse_s_sum - diag_sum) / N)
    return np.float32(loss)


# revision 44
# speedup vs baseline: 1.0648x; 1.0648x over previous
"""CLIP loss kernel for trn2, 8 NeuronCores, data-parallel over the batch dim.

Strategy (per core c of 8, SPMD):
  inputs: img slice [1024, 512] f32, spec slice [1024, 512] f32 (rows
  1024c..1024c+1023 of each modality).
  1. sumsq of both slices on ACT (Square + accum_out); 16/|row| =
     Sqrt(256 * reciprocal(ss)) (DVE reciprocal + ACT Sqrt). ACT uses
     exactly two table sets (sqrt preamble / exp main loop), both
     loaded off the critical path via warm-up activations.
  2. both modalities normalized on-device to bf16 (x16 fp8 prescale
     folded into the Sqrt scale), transposed via PE, cast to fp8 in the
     PSUM->SBUF staging copies. spec^T AllGathered in two chunks (chunk
     0 gates only the first half of the main loop; the first mesh
     cannot begin before the CC core's ~50-65us init anyway). The
     collective payload keeps rows = SBUF partitions, and specT uses an
     arrival-major layout [q, src core, k, off], so every unpack DMA
     moves 2KB-contiguous runs on both sides (the column permutation is
     harmless: the host only ever sums over all columns).
  3. logits block: out[m=img rows, n=spec cols] = imgT.T @ specT, fp8
     DoubleRow (K=256 per pass), PSUM f32, tiles [128, 2048].
  4. ACT Exp with constant scale = logit_scale/256; accum_out gives
     row-sums of exp for free. m=0 exp tile writes straight into
     racc[128, 8192] (bf16 column partials); m>0 tiles accumulated via
     DVE add. Final 128-partition column reduce happens on host.
  5. diag: raw img.spec dot per row (DVE, in the collective's shadow),
     combined with norms on host.
Host: gathers per-core row-sums / column partials / diag pieces, takes
logs and means (O(N) numpy) -> scalar loss.
"""

import os
from contextlib import ExitStack

import numpy as np

import concourse.bass as bass
import concourse.mybir as mybir
from concourse import bacc, tile
from concourse.bass_utils import run_bass_kernel_spmd
from concourse.masks import make_identity

N, D, C = 8192, 512, 8
NL = N // C  # 1024 local rows per core
P = 128
T = NL // P  # 8 natural [128, 512] tiles per modality slice
KC = D // P  # 4 contraction chunks
G = 4        # column groups per core block
GW = N // G  # 2048 columns per group

f32 = mybir.dt.float32
bf16 = mybir.dt.bfloat16
fp8 = mybir.dt.float8e4
FA = mybir.ActivationFunctionType
ALU = mybir.AluOpType

# fp8 operands are pre-scaled by 16 to stay out of the subnormal range;
# both sides carry the factor, compensated by scale/256 in the exp.
FP8_PRESCALE = 16.0

_cache: dict = {}

# dev bisection knobs (shipping defaults; NOTE tensor_tensor_reduce and
# fp8 PE transpose both fail on hardware — do not reintroduce them)
_USE_FP8T = os.environ.get("KERNEL_FP8T", "0") == "1"
_USE_DUMMY_CC = os.environ.get("KERNEL_DUMMY_CC", "0") == "1"


def _build(scale: float, use_cc: bool = True):
    nc = bacc.Bacc("TRN2", target_bir_lowering=False, debug=False, num_devices=C)
    img = nc.dram_tensor("img", [NL, D], bf16, kind="ExternalInput")
    spec = nc.dram_tensor("spec", [NL, D], bf16, kind="ExternalInput")
    rowsum_o = nc.dram_tensor("rowsum", [P, T], f32, kind="ExternalOutput")
    racc_o = nc.dram_tensor("racc_o", [P, N], bf16, kind="ExternalOutput")
    dotd_o = nc.dram_tensor("dotd", [P, T], f32, kind="ExternalOutput")
    rni_o = nc.dram_tensor("rni", [P, T], f32, kind="ExternalOutput")
    rns_o = nc.dram_tensor("rns", [P, T], f32, kind="ExternalOutput")

    tdt = fp8 if _USE_FP8T else bf16

    with tile.TileContext(nc) as tc, ExitStack() as ctx:
        const = ctx.enter_context(tc.tile_pool(name="const", bufs=1))
        natp = ctx.enter_context(tc.tile_pool(name="nat", bufs=T))
        scp = ctx.enter_context(tc.tile_pool(name="scr", bufs=2))
        spn = ctx.enter_context(tc.tile_pool(name="specn", bufs=4))
        pers = ctx.enter_context(tc.tile_pool(name="pers", bufs=1))
        ps = ctx.enter_context(tc.tile_pool(name="ps", bufs=2, space="PSUM"))
        ep = ctx.enter_context(tc.tile_pool(name="e", bufs=4))
        dramp = ctx.enter_context(tc.tile_pool(name="dram", bufs=1, space="DRAM"))

        if use_cc and _USE_DUMMY_CC:
            # First instruction of the program: trigger a tiny collective so
            # the CC core's ~30-40us init pipeline starts immediately. The
            # first real mesh cannot begin before that init completes.
            dmy_i = dramp.tile([1, 128], fp8, name="dmy_i")
            dmy_o = dramp.tile([C, 128], fp8, addr_space="Shared", name="dmy_o")
            nc.gpsimd.collective_compute(
                "AllGather",
                ALU.bypass,
                replica_groups=[list(range(C))],
                ins=[dmy_i.opt()],
                outs=[dmy_o.opt()],
            )

        ident_f = const.tile([P, P], f32, name="identf")
        make_identity(nc, ident_f)
        ident_t = const.tile([P, P], tdt, name="identt")
        nc.vector.tensor_copy(ident_t, ident_f)

        imgT = pers.tile([P, T, KC, P], fp8, name="imgT")
        # gathered spec^T in arrival-major layout [q, src core, k, off]:
        # both unpack DMA sides are then 2KB-contiguous per partition
        specT = pers.tile([P, 2, C, KC, 512], fp8, name="specT")
        stage = [pers.tile([P, NL], fp8, name=f"stage{k}") for k in range(KC)]
        racc = pers.tile([P, N], bf16, name="racc")
        # one extra slot: the split second half of the very last tile
        rowacc = pers.tile([P, T, G + 1], f32, name="rowacc")
        ssi = pers.tile([P, T], f32, name="ssi")
        sss = pers.tile([P, T], f32, name="sss")
        rsi = pers.tile([P, T], f32, name="rsi")
        rss = pers.tile([P, T], f32, name="rss")
        rni16 = pers.tile([P, T], f32, name="rni16")
        rns16 = pers.tile([P, T], f32, name="rns16")
        dotd = pers.tile([P, T], f32, name="dotd")
        rows = pers.tile([P, T], f32, name="rows")

        # two chunked AllGathers: the mesh has a ~14us floor and meshes
        # serialize, but chunk 0 gates only the first two column groups.
        # Payload rows = SBUF partitions so the unpack runs are contiguous.
        cc_in = [dramp.tile([P, KC * 512], fp8, name=f"cc_in{q}") for q in range(2)]
        cc_out = [
            dramp.tile([C * P, KC * 512], fp8, addr_space="Shared", name=f"cc_out{q}")
            for q in range(2)
        ]

        # preload the sqrt table set before the first Square needs it
        warm = const.tile([P, 1], f32, name="actwarm")
        nc.vector.memset(warm, 1.0)
        nc.scalar.activation(warm, warm, FA.Sqrt)

        # ---- per chunk (2 tiles): load -> norms -> normalize(fp8) ->
        #      transpose -> DMA PSUM->cc_in -> AllGather.
        #      img loads are deferred so the spec DMAs go first.
        img_nat, spec_nat = [], [None] * T
        for th in range(2):
            hs = slice(4 * th, 4 * th + 4)
            for tt in range(4):
                t = 4 * th + tt
                st = natp.tile([P, D], bf16, tag="specnat")
                nc.sync.dma_start(st, spec.ap()[t * P : (t + 1) * P, :])
                spec_nat[t] = st
                s2 = scp.tile([P, D], f32, tag="scr")
                nc.scalar.activation(
                    s2, st, FA.Square, accum_out=sss[:, t : t + 1]
                )
            nc.vector.tensor_scalar_max(sss[:, hs], sss[:, hs], 1.0e-6)
            nc.vector.reciprocal(rss[:, hs], sss[:, hs])
            # 16/|row|: sqrt(256 * 1/ss)
            nc.scalar.activation(
                rns16[:, hs], rss[:, hs], FA.Sqrt, scale=FP8_PRESCALE**2
            )
            pt = ps.tile([P, 2048], tdt, tag="mm")
            for tt in range(4):
                t = 4 * th + tt
                sn = spn.tile([P, D], tdt, tag="specn")
                nc.vector.tensor_scalar_mul(sn, spec_nat[t], rns16[:, t : t + 1])
                for k in range(KC):
                    nc.tensor.transpose(
                        pt[:, 512 * k + 128 * tt : 512 * k + 128 * (tt + 1)],
                        sn[:, 128 * k : 128 * (k + 1)],
                        ident_t,
                    )
            for k in range(KC):
                nc.vector.tensor_copy(
                    stage[k][:, 512 * th : 512 * (th + 1)],
                    pt[:, 512 * k : 512 * (k + 1)],
                )
                nc.sync.dma_start(
                    cc_in[th][:, 512 * k : 512 * (k + 1)],
                    stage[k][:, 512 * th : 512 * (th + 1)],
                )
            if use_cc:
                nc.gpsimd.collective_compute(
                    "AllGather",
                    ALU.bypass,
                    replica_groups=[list(range(C))],
                    ins=[cc_in[th].opt()],
                    outs=[cc_out[th].opt()],
                )

        for t in range(T):
            it = natp.tile([P, D], bf16, tag="imgnat")
            nc.sync.dma_start(it, img.ap()[t * P : (t + 1) * P, :])
            img_nat.append(it)

        # ---- img norms + transpose (overlap the collective) ----
        for t in range(T):
            s1 = scp.tile([P, D], f32, tag="scr")
            nc.scalar.activation(
                s1, img_nat[t], FA.Square, accum_out=ssi[:, t : t + 1]
            )
        nc.vector.tensor_scalar_max(ssi, ssi, 1.0e-6)
        nc.vector.reciprocal(rsi, ssi)
        nc.scalar.activation(rni16, rsi, FA.Sqrt, scale=FP8_PRESCALE**2)
        for t in range(T):
            ig = spn.tile([P, D], tdt, tag="specn")
            nc.vector.tensor_scalar_mul(ig, img_nat[t], rni16[:, t : t + 1])
            pti = ps.tile([P, 512], tdt, tag="mm")
            for k in range(KC):
                nc.tensor.transpose(
                    pti[:, 128 * k : 128 * (k + 1)],
                    ig[:, 128 * k : 128 * (k + 1)],
                    ident_t,
                )
            # ACT does this copy: it idles during the collective window
            nc.scalar.copy(imgT[:, t, :, :], pti)

        # switch the ACT table set to exp while the collective runs.
        # Reading rni16 (written by the last Sqrt) pins this after the
        # sqrt-set activations so the scheduler cannot hoist it earlier.
        warm2 = const.tile([P, 1], f32, name="actwarm2")
        nc.scalar.activation(warm2, rni16[:, 0:1], FA.Exp, scale=-1.0)

        # diag dots on DVE fill the collective's shadow (raw operands)
        for t in range(T):
            s3 = scp.tile([P, D], f32, tag="scr")
            nc.vector.tensor_mul(out=s3, in0=img_nat[t], in1=spec_nat[t])
            nc.vector.reduce_sum(
                dotd[:, t : t + 1], s3, axis=mybir.AxisListType.X
            )
        # these outputs are final already — ship them in the dead window
        # instead of serializing behind the main loop's tail
        nc.sync.dma_start(dotd_o.ap(), dotd)
        nc.sync.dma_start(rni_o.ap(), rni16)
        nc.sync.dma_start(rns_o.ap(), rns16)

        # ---- load gathered spec^T. specT[:, q, r, k, off] <-> global spec
        # row 1024*r + 512*q + off (host only ever sums over all columns, so
        # the column permutation needs no host-side handling). One DMA per
        # (q, r): 2KB-contiguous runs on both sides.
        for q in range(2):
            for r in range(C):
                if use_cc:
                    src = cc_out[q][P * r : P * (r + 1), :]
                else:  # debug: replicate the local slice (numerically wrong)
                    src = cc_in[q][:, :]
                nc.sync.dma_start(specT[:, q, r, :, :], src)

        # ---- main loop: logits block, exp, row/col accumulation ----
        nc.vector.memset(rowacc[:, :, G : G + 1], 0.0)
        escale = scale / (FP8_PRESCALE * FP8_PRESCALE)
        with nc.allow_low_precision("bf16 exp-sum accumulation, error ~0.5% -> <1e-3 on loss"):
            for g in range(G):
                gsl = racc[:, GW * g : GW * (g + 1)]
                for m in range(T):
                    pm = ps.tile([P, GW], f32, tag="mm")
                    # fp8 DoubleRow: each matmul contracts 2 k-chunks (K=256)
                    for q in range(KC // 2):
                        for ns in range(GW // 512):
                            b = 4 * g + ns  # 512-col block = (qq, r) of specT
                            nc.tensor.matmul(
                                pm[:, 512 * ns : 512 * (ns + 1)],
                                imgT[:, m, 2 * q : 2 * q + 2, :],
                                specT[:, b // C, b % C, 2 * q : 2 * q + 2, :],
                                start=(q == 0),
                                stop=(q == KC // 2 - 1),
                                perf_mode=mybir.MatmulPerfMode.DoubleRow,
                            )
                    if m == 0:
                        nc.scalar.activation(
                            gsl, pm, FA.Exp,
                            scale=escale,
                            accum_out=rowacc[:, m, g : g + 1],
                        )
                    elif g == G - 1 and m == T - 1:
                        # last tile: exp/add/store pipeline in halves so the
                        # kernel tail is ~2us shorter
                        for h in range(2):
                            hsl = slice(1024 * h, 1024 * (h + 1))
                            e = ep.tile([P, 1024], bf16, tag="e")
                            nc.scalar.activation(
                                e, pm[:, hsl], FA.Exp,
                                scale=escale,
                                accum_out=rowacc[:, m, g + h : g + h + 1],
                            )
                            nc.vector.tensor_add(
                                out=gsl[:, hsl], in0=gsl[:, hsl], in1=e
                            )
                            nc.sync.dma_start(
                                racc_o.ap()[:, GW * g + 1024 * h :
                                            GW * g + 1024 * (h + 1)],
                                gsl[:, hsl],
                            )
                    else:
                        e = ep.tile([P, GW], bf16, tag="e")
                        nc.scalar.activation(
                            e, pm, FA.Exp,
                            scale=escale,
                            accum_out=rowacc[:, m, g : g + 1],
                        )
                        nc.vector.tensor_add(out=gsl, in0=gsl, in1=e)
                if not (g == G - 1):
                    # racc[g] complete: ship it out now, overlapping next g
                    nc.sync.dma_start(
                        racc_o.ap()[:, GW * g : GW * (g + 1)], gsl
                    )

        # ---- tails ----
        nc.vector.reduce_sum(rows, rowacc[:, :, :], axis=mybir.AxisListType.X)
        nc.sync.dma_start(rowsum_o.ap(), rows)

    nc.compile()
    return nc


def _ensure_ntff_hook():
    """antenv.axon_hooks is absent on this image; provide the tiny get/set
    registry and register trn_agent_boot's ctypes NTFF hook so trace=True
    works. Only used from test runs (KERNEL_TRACE=1)."""
    import sys
    import types

    try:
        import antenv.axon_hooks  # noqa: F401
        return
    except ImportError:
        pass
    mod = types.ModuleType("antenv.axon_hooks")
    _state = {"hook": None}
    mod.set_axon_ntff_profile_hook = lambda h: _state.__setitem__("hook", h)
    mod.get_axon_ntff_profile_hook = lambda: _state["hook"]
    import antenv

    sys.modules["antenv.axon_hooks"] = mod
    antenv.axon_hooks = mod
    try:
        from trn_agent_boot.trn_boot import _ntff_profile_via_ctypes

        mod.set_axon_ntff_profile_hook(
            _ntff_profile_via_ctypes("/opt/axon/libaxon_pjrt.so")
        )
    except Exception as e:  # degrade to no tracing
        print(f"NTFF hook setup failed: {e}")


def kernel(image_features, spectrum_features, logit_scale):
    scale = float(np.asarray(logit_scale))
    key = round(scale, 9)
    if key not in _cache:
        _cache[key] = _build(scale)
    nc = _cache[key]

    import ml_dtypes

    img = np.ascontiguousarray(
        np.asarray(image_features, dtype=np.float32).astype(ml_dtypes.bfloat16)
    )
    spec = np.ascontiguousarray(
        np.asarray(spectrum_features, dtype=np.float32).astype(ml_dtypes.bfloat16)
    )
    in_maps = [
        {"img": img[c * NL : (c + 1) * NL], "spec": spec[c * NL : (c + 1) * NL]}
        for c in range(C)
    ]
    trace = os.environ.get("KERNEL_TRACE") == "1"
    if trace:
        _ensure_ntff_hook()
    res = run_bass_kernel_spmd(nc, in_maps, core_ids=list(range(C)), trace=trace)
    if trace:
        print(f"HW exec time: {res.exec_time_ns} ns (mean {res.mean_exec_time_ns})")

    rs = np.stack([r["rowsum"] for r in res.results]).astype(np.float64)   # [C,P,T]
    cs = np.stack(
        [r["racc_o"].astype(np.float64).sum(axis=0) for r in res.results]
    )  # [C,N]
    dd = np.stack([r["dotd"] for r in res.results]).astype(np.float64)
    ri = np.stack([r["rni"] for r in res.results]).astype(np.float64)
    rr = np.stack([r["rns"] for r in res.results]).astype(np.float64)

    # rni/rns outputs carry the x16 fp8 prescale each
    diag_sum = float(np.sum(scale * dd * ri * rr)) / (FP8_PRESCALE * FP8_PRESCALE)
    lse_i_sum = float(np.sum(np.log(rs)))
    col_total = cs.sum(axis=0)  # still in device (chunk-major) column order
    lse_s_sum = float(np.sum(np.log(col_total)))
    loss = 0.5 * ((lse_i_sum - diag_sum) / N + (lse_s_sum - diag_sum) / N)
    return np.float32(loss)
